# revision 11
# baseline (speedup 1.0000x reference)
"""Trainium2 Bass kernel for nn_KANLayer (Jacobi-polynomial KAN layer).

Math restructure
----------------
reference computes, per batch row b and output o:
    out[b,o] = mean_i( resid_scale[i]*tanh(x[b,i])
                       + spline_scale[i,o] * sum_c P_c(tanh(x[b,i])) * coefs[i,o,c] )
where P_c are Jacobi polynomials (alpha=beta=tanh(alpha_arctanh)) of degree c<=7.

Since P_c(t) = sum_k M[c,k] t^k with scalar coefficients M (depending only on
alpha), the whole layer collapses to

    out = b0 + sum_{k=1..7} tanh(x)^k @ Wk          (Wk: [IN, OUT])

with Wk, b0 folded on the host from coefs/spline_scale/resid_scale/M (the
resid branch folds into W1, the k=0 term into the bias b0).  The device then
only computes: tanh, 6 elementwise powers, and a [B,1792]x[1792,256] matmul.

Sharding: data-parallel over the batch dim across 8 cores (512 rows/core);
weights replicated.  Inputs are laid out host-side so the contraction dim
(i) sits on SBUF partitions — the device does no transposes at all:
  xt[p, n*512+b]   = x[c*512+b, n*128+p]              (x^T shard, packed)
  wd[p, s*128+m]   = Wfull[j*128+p, h*128+m], s=2j+h  (matmul-slot order)
  wd[p, 3584+h]    = b0[h*128+p]                      (bias columns)
  outT[h, p, b]    = out[c*512+b, h*128+p]            (output, transposed)
"""

import os
from contextlib import ExitStack

import numpy as np

import concourse.bacc as bacc
import concourse.tile as tile
from concourse import mybir
from concourse import bass_utils

B, IN, OUT, NCOEF = 4096, 256, 256, 8
NCORES = 8
BS = B // NCORES          # 512 batch rows per core
NK = 7                    # powers t^1..t^7
NJ = 2 * NK               # 14 K-chunks of 128 (contraction = 7*256)
NSLOT = 2 * NJ            # 28 matmuls (2 output halves)
WCOLS = NSLOT * 128       # 3584
F32 = mybir.dt.float32

# Matmul input dtype: float32r streams at 1 cycle/row (vs 4 for float32) and
# is bit-identical fp32 storage; numerics validated against the reference.
MM_DT = {
    "fp32": mybir.dt.float32,
    "f32r": mybir.dt.float32r,
    "bf16": mybir.dt.bfloat16,
}[os.environ.get("KAN_MM_DT", "f32r")]


def _emit_body(tc, xt_ap, wd_ap, outT_ap, mm_dt, rep=0):
    """One full per-core computation.  rep tags tile names for timing builds
    that replicate the body."""
    nc = tc.nc
    sfx = f"_r{rep}"

    ctx = ExitStack()
    io = ctx.enter_context(tc.tile_pool(name=f"io{sfx}", bufs=1))
    wp = ctx.enter_context(tc.tile_pool(name=f"wp{sfx}", bufs=1))
    pp = ctx.enter_context(tc.tile_pool(name=f"pp{sfx}", bufs=2, space="PSUM"))

    # ---- input DMAs -----------------------------------------------------
    xt_t = io.tile([128, 2 * BS], F32, tag=f"xt{sfx}")
    nc.sync.dma_start(out=xt_t, in_=xt_ap)

    # W in 7 chunks of 512 cols so matmuls can start as soon as their chunk
    # lands; chunk c covers matmul slots 4c..4c+3.  wd is declared with the
    # matmul dtype, so this is a byte-copy of pre-converted host data.
    wts = []
    for c in range(7):
        wt = wp.tile([128, 512], mm_dt, tag=f"w{c}{sfx}", name=f"w{c}{sfx}")
        nc.sync.dma_start(out=wt, in_=wd_ap[:, c * 512:(c + 1) * 512])
        wts.append(wt)
    bias_src = wd_ap[:, WCOLS:WCOLS + 2]
    if mm_dt != F32:
        bias_src = bias_src.bitcast(F32)
    bias_t = wp.tile([128, 2], F32, tag=f"bias{sfx}")
    nc.sync.dma_start(out=bias_t, in_=bias_src)

    # ---- powers of tanh(x): ACT does tanh + squares, DVE the odd muls ---
    # Tiles carry the matmul dtype so each producer writes properly rounded
    # values (the BIR verifier requires fp32r matmul inputs to be rounded).
    pows = [io.tile([128, 2 * BS], mm_dt, tag=f"t{k}{sfx}", name=f"t{k}{sfx}")
            for k in range(1, 8)]
    nc.scalar.activation(out=pows[0], in_=xt_t,
                         func=mybir.ActivationFunctionType.Tanh)
    nc.scalar.square(out=pows[1], in_=pows[0])            # t^2
    nc.vector.tensor_mul(pows[2], pows[0], pows[1])       # t^3
    nc.scalar.square(out=pows[3], in_=pows[1])            # t^4
    nc.vector.tensor_mul(pows[4], pows[1], pows[2])       # t^5
    nc.scalar.square(out=pows[5], in_=pows[2])            # t^6
    nc.vector.tensor_mul(pows[6], pows[2], pows[3])       # t^7

    # ---- 28 accumulating matmuls: out^T[h] = sum_j W_jh^T @ T_j ---------
    ps = [pp.tile([128, BS], F32, tag=f"ps{sfx}", name=f"ps{h}{sfx}")
          for h in range(2)]
    for j in range(NJ):
        k, n = j // 2, j % 2          # power index (0-based), i-chunk
        rhs = pows[k][:, n * BS:(n + 1) * BS]
        for h in range(2):
            s = 2 * j + h
            lhsT = wts[s // 4][:, (s % 4) * 128:(s % 4 + 1) * 128]
            nc.tensor.matmul(ps[h], lhsT=lhsT, rhs=rhs,
                             start=(j == 0), stop=(j == NJ - 1))

    # ---- bias add (DVE, reads PSUM) + store -----------------------------
    for h in range(2):
        o_t = io.tile([128, BS], F32, tag=f"o{h}{sfx}")
        nc.vector.tensor_scalar_add(o_t, ps[h], bias_t[:, h:h + 1])
        nc.sync.dma_start(out=outT_ap[h], in_=o_t)

    ctx.close()


def build_nc(mm_dt=MM_DT, reps=1):
    """Build the Bass module.  reps>1 replicates the body (same in/out
    tensors) for wall-clock HW timing via run-time deltas."""
    nc = bacc.Bacc("TRN2", target_bir_lowering=False, debug=False)
    xt = nc.dram_tensor("xt", [128, 2 * BS], F32, kind="ExternalInput")
    # wd carries the matmul dtype (f32r is fp32-layout, host data unchanged)
    wd = nc.dram_tensor("wd", [128, WCOLS + 2], mm_dt, kind="ExternalInput")
    outT = nc.dram_tensor("outT", [2, 128, BS], F32, kind="ExternalOutput")
    with tile.TileContext(nc) as tc:
        for r in range(reps):
            _emit_body(tc, xt.ap(), wd.ap(), outT.ap(), mm_dt, rep=r)
    nc.compile()
    return nc


def _jacobi_coef_matrix(alpha: float, n: int) -> np.ndarray:
    """M[c,k]: P_c(t) = sum_k M[c,k] t^k for Jacobi polys with alpha=beta."""
    M = np.zeros((n, n), dtype=np.float64)
    M[0, 0] = 1.0
    if n > 1:
        M[1, 1] = alpha + 1.0
    for m in range(2, n):
        c = 2.0 * m + 2.0 * alpha
        A = 2.0 * m * (m + 2.0 * alpha) * (c - 2.0)
        a_m = (c - 1.0) * c * (c - 2.0) / A
        b_m = 2.0 * (m + alpha - 1.0) ** 2 * c / A
        M[m, 1:] += a_m * M[m - 1, :-1]
        M[m, :] -= b_m * M[m - 2, :]
    return M


def fold_inputs(x, coefs, alpha_arctanh, resid_scale, spline_scale):
    """Host-side prep: fold params into (per-core xt shards, shared wd)."""
    x = np.ascontiguousarray(np.asarray(x, dtype=np.float32))
    alpha = float(np.tanh(np.float32(alpha_arctanh)))
    M = _jacobi_coef_matrix(alpha, NCOEF)
    C2 = (np.asarray(spline_scale, np.float64)[:, :, None]
          * np.asarray(coefs, np.float64) / IN)            # [i, o, c]
    Wk = np.einsum("ck,ioc->kio", M, C2)                   # [8, IN, OUT]
    b0 = Wk[0].sum(axis=0)                                 # [OUT]
    Wk[1] += np.asarray(resid_scale, np.float64) / IN      # resid branch
    Wfull = Wk[1:].reshape(NK * IN, OUT)                   # [(k-1)*IN+i, o]

    # wd[p, (2j+h)*128+m] = Wfull[j*128+p, h*128+m]; bias in last 2 cols
    wd = Wfull.reshape(NJ, 128, 2, 128).transpose(1, 0, 2, 3).reshape(128, WCOLS)
    if MM_DT == mybir.dt.float32r and os.environ.get("KAN_W_RNE", "0") == "1":
        # PE reads f32r (tf32: 10-bit mantissa); pre-round W with RNE on the
        # host so the load-time truncation doesn't bias the products.
        u = wd.astype(np.float32).view(np.uint32)
        u = (u + np.uint32(0xFFF) + ((u >> np.uint32(13)) & np.uint32(1))) \
            & np.uint32(0xFFFFE000)
        wd = u.view(np.float32).astype(np.float64)
    wd = np.concatenate([wd, np.stack([b0[:128], b0[128:]], axis=1)],
                        axis=1).astype(np.float32)
    wd = np.ascontiguousarray(wd)

    # xt[c][p, n*BS+b] = x[c*BS+b, n*128+p]
    xts = x.reshape(NCORES, BS, 2, 128).transpose(0, 3, 2, 1).reshape(
        NCORES, 128, 2 * BS)
    return [np.ascontiguousarray(xts[c]) for c in range(NCORES)], wd


def unshard_output(results):
    """results[c]['outT'] is [2, 128, BS]; rebuild [B, OUT]."""
    out = np.empty((B, OUT), dtype=np.float32)
    for c in range(NCORES):
        oT = results[c]["outT"]
        out[c * BS:(c + 1) * BS] = oT.transpose(2, 0, 1).reshape(BS, OUT)
    return out


_NC_CACHE = {}


def _get_nc(reps=1):
    key = (MM_DT, reps)
    if key not in _NC_CACHE:
        _NC_CACHE[key] = build_nc(MM_DT, reps)
    return _NC_CACHE[key]


def run(inputs, reps=1, **spmd_kwargs):
    """Shard, execute on 8 cores, unshard.  Returns (out, BassKernelResults)."""
    xts, wd = fold_inputs(**inputs)
    nc = _get_nc(reps)
    in_maps = [{"xt": xts[c], "wd": wd} for c in range(NCORES)]
    res = bass_utils.run_bass_kernel_spmd(
        nc, in_maps, core_ids=list(range(NCORES)), **spmd_kwargs)
    return unshard_output(res.results), res


def kernel(x, coefs, alpha_arctanh, resid_scale, spline_scale):
    out, _ = run(dict(x=x, coefs=coefs, alpha_arctanh=alpha_arctanh,
                      resid_scale=resid_scale, spline_scale=spline_scale))
    return out
